# revision 1
# baseline (speedup 1.0000x reference)
"""CompressAttn Trainium2 Bass kernel.

Problem: compressed-block attention.
  B=2, N=4096, QH=32, KH=2, D=VD=128, KSZ=32, STRIDE=16, M=255 blocks.
  kc[b,m,h,:] = sum_i w_k[i] * (k[b,16m+i,h,:] + pe_k[i,:])   (same for v)
  out = softmax(q @ kc^T * D^-0.5, causal-banded mask) @ vc, zero for n < 31.

Sharding: 8 cores = (batch b in {0,1}) x (query-head quarter hq in {0..3}).
Each core handles 8 query heads that share a single KV head (g = hq//2), so
K/V compression is done once per core.  No collectives needed; host gathers.

Per-core device pipeline (all matmuls in float32r = full-rate fp32 path):
  1. Compression via banded matmul: for each 128-row chunk c of k (stationary)
     stream a constant [128,16] block-diag weight tile -> psum [d, (a,t)] with
     P_a[t] = sum_{s<16} w[16a+s] k[16t+s].  kcT[d,m] = P0[m]+P1[m+1]+bias_k.
     v likewise -> vcT, then PE-transpose to natural vc and append a ones
     column (PV then yields the softmax denominator for free).
  2. Per (head, 512-col block b): scoresT[m-chunk, 512] = kcT_chunk^T @ qT.
  3. exp on ScalarE (scale = D^-0.5 fused), multiplicative static staircase
     mask on the 32 diagonal band rows (b-independent [32,512] constant).
  4. Per 128-query tile: PV = expT_tile^T @ [vc|1|vc|1] (258 cols to stay on
     the fast fp32r path), accumulate 2 m-chunks in psum.  reciprocal of the
     ones column, ScalarE per-partition scale -> natural [n, vd] output tile.
"""

import ml_dtypes
import numpy as np

import concourse.bacc as bacc
import concourse.mybir as mybir
import concourse.tile as tile
from concourse.bass_utils import run_bass_kernel_spmd

# Problem geometry (hardcoded per contest rules).
B, N, QH, KH, D, VD = 2, 4096, 32, 2, 128, 128
KSZ, STRIDE = 32, 16
M = (N - KSZ) // STRIDE + 1          # 255 compressed blocks
T16 = N // STRIDE                     # 256 stride-16 sub-blocks
HPC = QH // 4                         # 8 query heads per core
NBLK = N // 512                       # 8 query blocks of 512
SM = float(D) ** -0.5

F32 = mybir.dt.float32
F32R = mybir.dt.float32r
BF16 = mybir.dt.bfloat16

# dtype switches for the two big matmul stages (float32r = single-pass fp32)
QK_DT = BF16
PV_DT = BF16
CP_DT = F32R


def _r(ap, dt):
    return ap


def build_program():
    nc = bacc.Bacc("TRN2", target_bir_lowering=False, debug=False)

    qT_d = nc.dram_tensor("qT", [HPC, D, N], QK_DT, kind="ExternalInput")
    k_d = nc.dram_tensor("kk", [N, D], CP_DT, kind="ExternalInput")
    v_d = nc.dram_tensor("vv", [N, D], CP_DT, kind="ExternalInput")
    w01k_d = nc.dram_tensor("w01k", [128, 16], CP_DT, kind="ExternalInput")
    w01v_d = nc.dram_tensor("w01v", [128, 16], CP_DT, kind="ExternalInput")
    bk_d = nc.dram_tensor("biask", [128, 1], F32, kind="ExternalInput")
    bv_d = nc.dram_tensor("biasv", [128, 1], F32, kind="ExternalInput")
    m01_d = nc.dram_tensor("m01", [8, 128, 512], PV_DT, kind="ExternalInput")
    id_d = nc.dram_tensor("ident", [128, 128], F32, kind="ExternalInput")
    ones_d = nc.dram_tensor("ones1", [128, 2], PV_DT, kind="ExternalInput")
    o_d = nc.dram_tensor("o", [HPC, N, VD], F32, kind="ExternalOutput")

    with tile.TileContext(nc) as tc:
        with tc.tile_pool(name="consts", bufs=1) as cp:
            w01k = cp.tile([128, 16], CP_DT)
            w01v = cp.tile([128, 16], CP_DT)
            biask = cp.tile([128, 1], F32)
            biasv = cp.tile([128, 1], F32)
            m01 = cp.tile([128, 8 * 512], PV_DT)
            ident = cp.tile([128, 128], F32)
            ktile = cp.tile([128, 32 * 128], CP_DT)
            vtile = cp.tile([128, 32 * 128], CP_DT)
            kcT = cp.tile([128, M], QK_DT)        # [d, m]
            vcT = cp.tile([128, 256], F32)      # [d, t] staging
            vca0 = cp.tile([128, 130], PV_DT)     # [m 0:128,   vc|1|0]
            vca1 = cp.tile([128, 130], PV_DT)     # [m 128:255, vc|1|0]

            nc.sync.dma_start(w01k[:, :], w01k_d.ap())
            nc.sync.dma_start(w01v[:, :], w01v_d.ap())
            nc.sync.dma_start(biask[:, :], bk_d.ap())
            nc.sync.dma_start(biasv[:, :], bv_d.ap())
            nc.sync.dma_start(
                m01[:, :].rearrange("p (j n) -> p j n", j=8),
                m01_d.ap().rearrange("j p n -> p j n"),
            )
            nc.sync.dma_start(ident[:, :], id_d.ap())
            nc.sync.dma_start(
                ktile[:, :].rearrange("p (c d) -> p c d", c=32),
                k_d.ap().rearrange("(c r) d -> r c d", r=128),
            )
            nc.sync.dma_start(
                vtile[:, :].rearrange("p (c d) -> p c d", c=32),
                v_d.ap().rearrange("(c r) d -> r c d", r=128),
            )

            # ---- compression ----
            with tc.tile_pool(name="ppsum", bufs=1, space="PSUM") as pp:
                # free layout (t, a): pkT[d, 2t+a] = P_a[t]
                pkT = pp.tile([128, 512], F32)
                pvT = pp.tile([128, 512], F32)
                tpA = pp.tile([128, 128], F32)
                tpB = pp.tile([128, 128], F32)
                for c in range(32):
                    nc.tensor.matmul(
                        pkT[:, 16 * c : 16 * c + 16],
                        _r(ktile[:, 128 * c : 128 * (c + 1)], CP_DT),
                        _r(w01k[:, :], CP_DT),
                        start=True, stop=True,
                    )
                    nc.tensor.matmul(
                        pvT[:, 16 * c : 16 * c + 16],
                        _r(vtile[:, 128 * c : 128 * (c + 1)], CP_DT),
                        _r(w01v[:, :], CP_DT),
                        start=True, stop=True,
                    )
                # kcT[d,m] = P0[m] + P1[m+1] + bias_k[d]
                pk3 = pkT[:, :].rearrange("p (t a) -> p t a", a=2)
                pv3 = pvT[:, :].rearrange("p (t a) -> p t a", a=2)
                # (walrus: only one PSUM input per DVE op -> two steps)
                nc.vector.tensor_scalar_add(kcT[:, 0:M], pk3[:, 0:M, 0], biask[:, 0:1])
                nc.vector.tensor_add(kcT[:, 0:M], kcT[:, 0:M], pk3[:, 1 : M + 1, 1])
                nc.vector.tensor_scalar_add(vcT[:, 0:M], pv3[:, 0:M, 0], biasv[:, 0:1])
                nc.vector.tensor_add(vcT[:, 0:M], vcT[:, 0:M], pv3[:, 1 : M + 1, 1])
                nc.vector.memset(vcT[:, M : M + 1], 0.0)
                # transpose vcT -> natural vc, build [vc|1|vc|1]
                nc.tensor.transpose(tpA[:, :], vcT[:, 0:128], ident[:, :])
                nc.tensor.transpose(tpB[:, :], vcT[:, 128:256], ident[:, :])
                nc.vector.tensor_copy(vca0[:, 0:128], tpA[:, :])
                nc.vector.tensor_copy(vca1[:, 0:128], tpB[:, :])
                nc.sync.dma_start(vca0[:, 128:130], ones_d.ap())
                nc.sync.dma_start(vca1[:, 128:130], ones_d.ap())

            # ---- attention ----
            with (
                tc.tile_pool(name="qp", bufs=2) as qp,
                tc.tile_pool(name="ep", bufs=4) as ep,
                tc.tile_pool(name="op", bufs=2) as op,
                tc.tile_pool(name="rp", bufs=8) as rp,
                tc.tile_pool(name="sps", bufs=4, space="PSUM") as sps,
                tc.tile_pool(name="pvs", bufs=2, space="PSUM") as pvs,
            ):
                for h in range(HPC):
                    qTh = qp.tile([128, N], QK_DT, tag="qTh")
                    nc.sync.dma_start(qTh[:, :], qT_d.ap()[h])
                    for b in range(NBLK):
                        mr = min(32 * b + 31, M)      # visible m count
                        c0r = min(mr, 128)
                        c1r = mr - 128
                        qs = qTh[:, 512 * b : 512 * (b + 1)]

                        sT0 = sps.tile([128, 512], F32, tag="sT")
                        nc.tensor.matmul(
                            sT0[0:c0r, :],
                            _r(kcT[:, 0:c0r], QK_DT),
                            _r(qs, QK_DT),
                            start=True, stop=True,
                        )
                        eT0 = ep.tile([128, 512], PV_DT, tag="eT")
                        nc.scalar.activation(
                            eT0[0:c0r, :], sT0[0:c0r, :],
                            mybir.ActivationFunctionType.Exp, scale=SM,
                        )
                        if c1r > 0:
                            sT1 = sps.tile([128, 512], F32, tag="sT")
                            nc.tensor.matmul(
                                sT1[0:c1r, :],
                                _r(kcT[:, 128 : 128 + c1r], QK_DT),
                                _r(qs, QK_DT),
                                start=True, stop=True,
                            )
                            eT1 = ep.tile([128, 512], PV_DT, tag="eT")
                            nc.scalar.activation(
                                eT1[0:c1r, :], sT1[0:c1r, :],
                                mybir.ActivationFunctionType.Exp, scale=SM,
                            )
                        # multiplicative staircase mask over the aligned
                        # 64-row window [32b-32, 32b+32); m01 row r covers
                        # m = 32b-32+r (visible iff n' >= 16r-481).
                        # staircase mask variant v holds stair[p-32v+32] at
                        # partition p, so both operands share base partitions
                        # (32-row pieces: non-zero-base APs cap at 32 rows).
                        w0 = 32 * b - 32
                        for ww in (w0, w0 + 32):
                            s0, e0 = max(ww, 0), min(ww + 32, c0r)
                            if s0 < e0:
                                mj = m01[:, 512 * b : 512 * (b + 1)]
                                nc.vector.tensor_mul(
                                    eT0[s0:e0, :], eT0[s0:e0, :], mj[s0:e0, :]
                                )
                            if c1r > 0:
                                s1 = max(ww, 128) - 128
                                e1 = min(ww + 32, 128 + c1r) - 128
                                if s1 < e1:
                                    mj = m01[:, 512 * (b - 4) : 512 * (b - 3)]
                                    nc.vector.tensor_mul(
                                        eT1[s1:e1, :], eT1[s1:e1, :], mj[s1:e1, :]
                                    )
                        o_blk = op.tile([128, 512], F32, tag="o")
                        for pr in range(2):
                            pvt = pvs.tile([128, 512], F32, tag="pv")
                            pv3 = pvt[:, 0:260].rearrange(
                                "p (j c) -> p j c", j=2
                            )
                            for j in range(2):
                                tt = 2 * pr + j
                                t = 4 * b + tt
                                K = 8 * t + 7
                                c0k = min(K, 128)
                                c1k = K - 128
                                out_ap = pvt[:, 130 * j : 130 * j + 130]
                                nc.tensor.matmul(
                                    out_ap,
                                    _r(eT0[0:c0k, 128 * tt : 128 * (tt + 1)], PV_DT),
                                    _r(vca0[0:c0k, :], PV_DT),
                                    start=True, stop=(c1k <= 0),
                                )
                                if c1k > 0:
                                    nc.tensor.matmul(
                                        out_ap,
                                        _r(eT1[0:c1k, 128 * tt : 128 * (tt + 1)], PV_DT),
                                        _r(vca1[0:c1k, :], PV_DT),
                                        start=False, stop=True,
                                    )
                            rc = rp.tile([128, 2], F32, tag="rc")
                            if b == 0 and pr == 0:
                                rtmp = rp.tile([128, 2], F32, tag="rtmp")
                                nc.vector.tensor_scalar_add(
                                    rtmp[:, :], pv3[:, :, 128], 1e-30
                                )
                                nc.vector.reciprocal(rc[:, :], rtmp[:, :])
                            else:
                                nc.vector.reciprocal(rc[:, :], pv3[:, :, 128])
                            for j in range(2):
                                tt = 2 * pr + j
                                dst = o_blk[:, 128 * tt : 128 * (tt + 1)]
                                src = pvt[:, 130 * j : 130 * j + 128]
                                if tt % 2 == 0:
                                    nc.scalar.mul(dst, src, rc[:, j : j + 1])
                                else:
                                    nc.vector.tensor_scalar_mul(
                                        dst, src, rc[:, j : j + 1]
                                    )
                        nc.sync.dma_start(
                            o_d.ap()[h, 512 * b : 512 * (b + 1), :].rearrange(
                                "(tt p) vd -> p tt vd", p=128
                            ),
                            o_blk[:, :].rearrange("p (tt vd) -> p tt vd", tt=4),
                        )
    nc.compile()
    return nc


def make_consts(w_k, pe_k, w_v, pe_v):
    """Host-side constant tensors fed to every core."""
    f = np.float32
    w01k = np.zeros((128, 16), f)
    w01v = np.zeros((128, 16), f)
    for r in range(128):
        j = r // 16
        s = r % 16
        for a in range(2):
            # column layout (j, a): col = 2*j + a, matching psum (t, a)
            w01k[r, 2 * j + a] = w_k[16 * a + s]
            w01v[r, 2 * j + a] = w_v[16 * a + s]
    biask = (w_k[:, None] * pe_k).sum(0).astype(f)[:, None]  # [128,1]
    biasv = (w_v[:, None] * pe_v).sum(0).astype(f)[:, None]
    # variant v: row p = stair(p - 32v + 32); stair(r): n' >= 16r - 481
    m01 = np.ones((8, 128, 512), f)
    for vv in range(8):
        for p in range(128):
            r = p - 32 * vv + 32
            if 0 <= r < 64:
                lo = 16 * r - 481
                if lo >= 512:
                    m01[vv, p, :] = 0.0
                else:
                    m01[vv, p, : max(lo, 0)] = 0.0
    ident = np.eye(128, dtype=f)
    return {
        "w01k": np.ascontiguousarray(w01k),
        "w01v": np.ascontiguousarray(w01v),
        "biask": np.ascontiguousarray(biask),
        "biasv": np.ascontiguousarray(biasv),
        "m01": m01.astype(ml_dtypes.bfloat16),
        "ident": ident,
        "ones1": np.hstack([np.ones((128, 1)), np.zeros((128, 1))]).astype(ml_dtypes.bfloat16),
    }


def make_in_map(q, k, v, consts, core):
    b, hq = core // 4, core % 4
    g = hq // 2
    qT = np.ascontiguousarray(
        q[b, :, 8 * hq : 8 * (hq + 1), :].transpose(1, 2, 0)
    ).astype(ml_dtypes.bfloat16)  # [8, D, N]
    return {
        "qT": qT,
        "kk": np.ascontiguousarray(k[b, :, g, :]),
        "vv": np.ascontiguousarray(v[b, :, g, :]),
        **consts,
    }


_CACHE = {}


def _compiled():
    if "nc" not in _CACHE:
        _CACHE["nc"] = build_program()
    return _CACHE["nc"]


def kernel(q, k, v, w_k, pe_k, w_v, pe_v, _trace=False, _trace_kwargs=None):
    q = np.asarray(q, np.float32)
    k = np.asarray(k, np.float32)
    v = np.asarray(v, np.float32)
    consts = make_consts(
        np.asarray(w_k, np.float32), np.asarray(pe_k, np.float32),
        np.asarray(w_v, np.float32), np.asarray(pe_v, np.float32),
    )
    nc = _compiled()
    in_maps = [make_in_map(q, k, v, consts, c) for c in range(8)]
    kw = {}
    if _trace:
        kw = {"trace": True, **(_trace_kwargs or {})}
    res = run_bass_kernel_spmd(nc, in_maps, core_ids=list(range(8)), **kw)
    out = np.empty((B, N, QH, VD), np.float32)
    for c in range(8):
        b, hq = c // 4, c % 4
        out[b, :, 8 * hq : 8 * (hq + 1), :] = res.results[c]["o"].transpose(1, 0, 2)
    _CACHE["last_result"] = res
    return out



# revision 5
# speedup vs baseline: 1.4022x; 1.4022x over previous
"""CompressAttn Trainium2 Bass kernel (v2: head-mixed tiles, host norm).

Problem: compressed-block attention.
  B=2, N=4096, QH=32, KH=2, D=VD=128, KSZ=32, STRIDE=16, M=255 blocks.
  kc[b,m,h,:] = sum_i w_k[i] * (k[b,16m+i,h,:] + pe_k[i,:])   (same for v)
  out = softmax(q @ kc^T * D^-0.5, causal-banded mask) @ vc, zero for n < 31.

Sharding: 8 cores = (batch b in {0,1}) x (query-head quarter hq in {0..3}).
Each core handles 8 query heads sharing one KV head; K/V compression done
once per core.  No collectives; host gathers + normalizes.

Device pipeline per 64-query tile t (q columns are head-mixed: 64 queries x
8 heads = 512 moving columns per matmul, so all matmuls stream 512 cols):
  1. QK:   sT[m_c, 512] = kcT_c^T @ q_t        (1-2 chunk matmuls -> psum)
  2. exp:  eT = exp(sm * sT) on ScalarE, one activation spanning both
           psum banks when 2 chunks.
  3. mask: 4 staircase rows (m = 4t-1..4t+2) multiplied by a single
           constant [128,512] mask tile on GpSimd (row p pattern:
           j >= 15+16*((p+1)%4), repeated per head).
  4. PV:   oT[vd, 512] += vc_c^T(natural, stationary) @ eT_c  (psum)
  5. den:  den[32 dup rows, 512] += ones32^T @ eT_c, 4 tiles per psum
           bank at partition 32*(t%4); one DVE copy drains 4 tiles.
  6. DVE copies psum -> SBUF bf16; DMA out.  Softmax division on HOST
     (out_unnorm / den, zero where den==0).
"""

import ml_dtypes
import numpy as np

import concourse.bacc as bacc
import concourse.mybir as mybir
import concourse.tile as tile
from concourse.bass_utils import run_bass_kernel_spmd

# Problem geometry (hardcoded).
B, N, QH, KH, D, VD = 2, 4096, 32, 2, 128, 128
KSZ, STRIDE = 32, 16
M = (N - KSZ) // STRIDE + 1          # 255 compressed blocks
HPC = QH // 4                         # 8 query heads per core
NT = 64                               # 64-query tiles per core
SM = float(D) ** -0.5

F32 = mybir.dt.float32
BF16 = mybir.dt.bfloat16
EXP = mybir.ActivationFunctionType.Exp


def build_program():
    nc = bacc.Bacc("TRN2", target_bir_lowering=False, debug=False)

    qT_d = nc.dram_tensor("qT", [128, NT * 512], BF16, kind="ExternalInput")
    k_d = nc.dram_tensor("kk", [N, D], BF16, kind="ExternalInput")
    v_d = nc.dram_tensor("vv", [N, D], BF16, kind="ExternalInput")
    w01k_d = nc.dram_tensor("w01k", [128, 16], BF16, kind="ExternalInput")
    w01v_d = nc.dram_tensor("w01v", [128, 16], BF16, kind="ExternalInput")
    bk_d = nc.dram_tensor("biask", [128, 1], F32, kind="ExternalInput")
    bv_d = nc.dram_tensor("biasv", [128, 1], F32, kind="ExternalInput")
    mk_d = nc.dram_tensor("maskc", [128, 9 * 512], BF16, kind="ExternalInput")
    id_d = nc.dram_tensor("ident", [128, 128], F32, kind="ExternalInput")
    oT_d = nc.dram_tensor("oT", [128, NT * 512], BF16, kind="ExternalOutput")
    den_d = nc.dram_tensor("den", [16, 128, 512], BF16, kind="ExternalOutput")

    with tile.TileContext(nc) as tc:
        with tc.tile_pool(name="consts", bufs=1) as cp:
            w01k = cp.tile([128, 16], BF16)
            w01v = cp.tile([128, 16], BF16)
            biask = cp.tile([128, 1], F32)
            biasv = cp.tile([128, 1], F32)
            maskc = cp.tile([128, 9 * 512], BF16)
            ident = cp.tile([128, 128], F32)
            ones32 = cp.tile([128, 32], BF16)
            ktile = cp.tile([128, 32 * 128], BF16)
            vtile = cp.tile([128, 32 * 128], BF16)
            kcT = cp.tile([128, M], BF16)         # [d, m]
            vcT = cp.tile([128, 256], F32)        # [d, m] staging
            vca0 = cp.tile([128, 128], BF16)      # vc natural, m 0:128
            vca1 = cp.tile([128, 128], BF16)      # vc natural, m 128:255(+pad)
            qsb = cp.tile([128, NT * 512], BF16)  # [d, (t h j)]

            nc.sync.dma_start(w01k[:, :], w01k_d.ap())
            nc.sync.dma_start(w01v[:, :], w01v_d.ap())
            nc.sync.dma_start(biask[:, :], bk_d.ap())
            nc.sync.dma_start(biasv[:, :], bv_d.ap())
            nc.sync.dma_start(maskc[:, :], mk_d.ap())
            nc.sync.dma_start(ident[:, :], id_d.ap())
            nc.vector.memset(ones32[:, :], 1.0)
            nc.sync.dma_start(
                ktile[:, :].rearrange("p (c d) -> p c d", c=32),
                k_d.ap().rearrange("(c r) d -> r c d", r=128),
            )
            nc.sync.dma_start(
                vtile[:, :].rearrange("p (c d) -> p c d", c=32),
                v_d.ap().rearrange("(c r) d -> r c d", r=128),
            )
            for s in range(8):
                nc.sync.dma_start(
                    qsb[:, 4096 * s : 4096 * (s + 1)],
                    qT_d.ap()[:, 4096 * s : 4096 * (s + 1)],
                )

            # ---- compression ----
            with tc.tile_pool(name="ppsum", bufs=1, space="PSUM") as pp:
                pkT = pp.tile([128, 512], F32)   # [d, (T a)]
                pvT = pp.tile([128, 512], F32)
                tpA = pp.tile([128, 128], F32)
                tpB = pp.tile([128, 128], F32)
                for c in range(32):
                    nc.tensor.matmul(
                        pkT[:, 16 * c : 16 * c + 16],
                        ktile[:, 128 * c : 128 * (c + 1)],
                        w01k[:, :],
                        start=True, stop=True,
                    )
                    nc.tensor.matmul(
                        pvT[:, 16 * c : 16 * c + 16],
                        vtile[:, 128 * c : 128 * (c + 1)],
                        w01v[:, :],
                        start=True, stop=True,
                    )
                # kcT[d,m] = P0[m] + P1[m+1] + bias_k[d]
                pk3 = pkT[:, :].rearrange("p (t a) -> p t a", a=2)
                pv3 = pvT[:, :].rearrange("p (t a) -> p t a", a=2)
                nc.vector.tensor_scalar_add(kcT[:, 0:M], pk3[:, 0:M, 0], biask[:, 0:1])
                nc.vector.tensor_add(kcT[:, 0:M], kcT[:, 0:M], pk3[:, 1 : M + 1, 1])
                nc.vector.tensor_scalar_add(vcT[:, 0:M], pv3[:, 0:M, 0], biasv[:, 0:1])
                nc.vector.tensor_add(vcT[:, 0:M], vcT[:, 0:M], pv3[:, 1 : M + 1, 1])
                nc.vector.memset(vcT[:, M : M + 1], 0.0)
                nc.tensor.transpose(tpA[:, :], vcT[:, 0:128], ident[:, :])
                nc.tensor.transpose(tpB[:, :], vcT[:, 128:256], ident[:, :])
                nc.vector.tensor_copy(vca0[:, :], tpA[:, :])
                nc.vector.tensor_copy(vca1[:, :], tpB[:, :])

            # ---- attention ----
            def tile_geom(t):
                ctot = 4 * t + 3            # visible m count (= min(.,255))
                c0 = min(ctot, 128)
                c1 = ctot - 128
                return c0, c1

            state = {}

            def emit_qk(t, sps):
                c0, c1 = tile_geom(t)
                nblk = 2 if c1 > 0 else 1
                sp = sps.tile([128, 512 * nblk], F32, tag=f"sp{nblk}")
                nc.tensor.matmul(
                    sp[0:c0, 0:512], kcT[:, 0:c0],
                    qsb[:, 512 * t : 512 * (t + 1)],
                    start=True, stop=True,
                )
                if c1 > 0:
                    nc.tensor.matmul(
                        sp[0:c1, 512:1024], kcT[:, 128 : 128 + c1],
                        qsb[:, 512 * t : 512 * (t + 1)],
                        start=True, stop=True,
                    )
                state[t] = sp

            def emit_exp_mask(t, ep):
                c0, c1 = tile_geom(t)
                nblk = 2 if c1 > 0 else 1
                sp = state.pop(t)
                eT = ep.tile([128, 512 * nblk], BF16, tag=f"eT{nblk}")
                nc.scalar.activation(eT[:, :], sp[:, :], EXP, scale=SM)
                # staircase mask rows m = 4t-1 .. 4t+2, applied on 32-aligned
                # windows (engine APs require 32-aligned partition bases).
                # variants: blk 0 = row r%32==31 (piece A), blk 1 = rows 0..2
                # (piece B), blk 1+v = rows 4v-1..4v+2 (v = t%8 in 1..7).
                v = t % 8
                if v == 0:
                    pieces = ([(4 * t - 1, 0)] if t > 0 else []) + [(4 * t, 1)]
                else:
                    pieces = [(4 * t - 1, 1 + v)]
                for row0, blk in pieces:
                    if row0 < 128:
                        a, coff = 32 * (row0 // 32), 0
                    else:
                        a, coff = 32 * ((row0 - 128) // 32), 512
                    nc.gpsimd.tensor_mul(
                        eT[a : a + 32, coff : coff + 512],
                        eT[a : a + 32, coff : coff + 512],
                        maskc[a : a + 32, 512 * blk : 512 * (blk + 1)],
                    )
                state[t] = eT

            def emit_pv_den(t, pvs, dnp, obp, dbp):
                c0, c1 = tile_geom(t)
                eT = state.pop(t)
                po = pvs.tile([128, 512], F32, tag="po")
                nc.tensor.matmul(
                    po[:, :], vca0[0:c0, :], eT[0:c0, 0:512],
                    start=True, stop=(c1 <= 0),
                )
                if c1 > 0:
                    nc.tensor.matmul(
                        po[:, :], vca1[0:c1, :], eT[0:c1, 512:1024],
                        start=False, stop=True,
                    )
                r = t % 4
                if r == 0:
                    state["dn"] = dnp.tile([128, 512], F32, tag="dn", name="dn")
                dn = state["dn"]
                nc.tensor.matmul(
                    dn[32 * r : 32 * r + 32, :], ones32[0:c0, :],
                    eT[0:c0, 0:512],
                    start=True, stop=(c1 <= 0), skip_group_check=True,
                    tile_position=(0, 32 * r),
                )
                if c1 > 0:
                    nc.tensor.matmul(
                        dn[32 * r : 32 * r + 32, :], ones32[0:c1, :],
                        eT[0:c1, 512:1024],
                        start=False, stop=True, skip_group_check=True,
                        tile_position=(0, 32 * r),
                    )
                # drain PV psum -> bf16 staging (pairs of tiles -> one DMA)
                if t % 2 == 0:
                    state["ob"] = obp.tile([128, 1024], BF16, tag="ob", name="ob")
                ob = state["ob"]
                nc.vector.tensor_copy(ob[:, 512 * (t % 2) : 512 * (t % 2 + 1)], po[:, :])
                if t % 2 == 1:
                    nc.sync.dma_start(
                        oT_d.ap()[:, 1024 * (t // 2) : 1024 * (t // 2 + 1)],
                        ob[:, :],
                    )
                if r == 3:
                    db = dbp.tile([128, 512], BF16, tag="db")
                    nc.vector.tensor_copy(db[:, :], dn[:, :])
                    nc.scalar.dma_start(den_d.ap()[t // 4], db[:, :])

            with (
                tc.tile_pool(name="ep", bufs=3) as ep,
                tc.tile_pool(name="obp", bufs=2) as obp,
                tc.tile_pool(name="dbp", bufs=2) as dbp,
                tc.tile_pool(name="pvs", bufs=2, space="PSUM") as pvs,
                tc.tile_pool(name="dns", bufs=2, space="PSUM") as dns,
            ):
                with tc.tile_pool(name="spsA", bufs=3, space="PSUM") as spsA:
                    for t in range(0, 32):
                        emit_qk(t, spsA)
                        if t > 0:
                            emit_pv_den(t - 1, pvs, dns, obp, dbp)
                        emit_exp_mask(t, ep)
                    emit_pv_den(31, pvs, dns, obp, dbp)
                with tc.tile_pool(name="spsB", bufs=2, space="PSUM") as spsB:
                    for t in range(32, 64):
                        emit_qk(t, spsB)
                        if t > 32:
                            emit_pv_den(t - 1, pvs, dns, obp, dbp)
                        emit_exp_mask(t, ep)
                    emit_pv_den(63, pvs, dns, obp, dbp)
    nc.compile()
    return nc


def make_consts(w_k, pe_k, w_v, pe_v):
    """Host-side constant tensors fed to every core."""
    f = np.float32
    w01k = np.zeros((128, 16), f)
    w01v = np.zeros((128, 16), f)
    for r in range(128):
        j = r // 16
        s = r % 16
        for a in range(2):
            w01k[r, 2 * j + a] = w_k[16 * a + s]
            w01v[r, 2 * j + a] = w_v[16 * a + s]
    biask = (w_k[:, None] * pe_k).sum(0).astype(f)[:, None]  # [128,1]
    biasv = (w_v[:, None] * pe_v).sum(0).astype(f)[:, None]
    # staircase masks on 32-aligned windows; queries are (h, j) blocks of
    # 64; staircase row with in-stair index delta is visible iff
    # j >= 15 + 16*delta.  blk 0: delta 0 at r==31; blk 1: delta 1..3 at
    # r==0..2; blk 1+v (v=1..7): delta 0..3 at r==4v-1..4v+2.
    maskc = np.ones((128, 9, 512), f)
    jj = np.arange(64)

    def stair(delta):
        return np.tile((jj >= 15 + 16 * delta).astype(f), 8)

    for p in range(128):
        r = p % 32
        if r == 31:
            maskc[p, 0] = stair(0)
        if r in (0, 1, 2):
            maskc[p, 1] = stair(r + 1)
        for v in range(1, 8):
            d = r - (4 * v - 1)
            if 0 <= d < 4:
                maskc[p, 1 + v] = stair(d)
    maskc = maskc.reshape(128, 9 * 512)
    ident = np.eye(128, dtype=f)
    bf = ml_dtypes.bfloat16
    return {
        "w01k": w01k.astype(bf),
        "w01v": w01v.astype(bf),
        "biask": biask,
        "biasv": biasv,
        "maskc": maskc.astype(bf),
        "ident": ident,
    }


def make_in_map(q, k, v, consts, core):
    b, hq = core // 4, core % 4
    g = hq // 2
    bf = ml_dtypes.bfloat16
    # qT layout [d, t, h, j]: q[b, 64t+j, 8hq+h, d]
    qc = q[b, :, 8 * hq : 8 * (hq + 1), :]            # [N, 8, 128]
    qT = np.ascontiguousarray(
        qc.reshape(64, 64, 8, 128).transpose(3, 0, 2, 1)
    ).reshape(128, -1).astype(bf)
    return {
        "qT": qT,
        "kk": np.ascontiguousarray(k[b, :, g, :]).astype(bf),
        "vv": np.ascontiguousarray(v[b, :, g, :]).astype(bf),
        **consts,
    }


_CACHE = {}


def _compiled():
    if "nc" not in _CACHE:
        _CACHE["nc"] = build_program()
    return _CACHE["nc"]


def kernel(q, k, v, w_k, pe_k, w_v, pe_v, _trace=False, _trace_kwargs=None):
    q = np.asarray(q, np.float32)
    k = np.asarray(k, np.float32)
    v = np.asarray(v, np.float32)
    consts = make_consts(
        np.asarray(w_k, np.float32), np.asarray(pe_k, np.float32),
        np.asarray(w_v, np.float32), np.asarray(pe_v, np.float32),
    )
    nc = _compiled()
    in_maps = [make_in_map(q, k, v, consts, c) for c in range(8)]
    kw = {}
    if _trace:
        kw = {"trace": True, **(_trace_kwargs or {})}
    res = run_bass_kernel_spmd(nc, in_maps, core_ids=list(range(8)), **kw)
    out = np.empty((B, N, QH, VD), np.float32)
    for c in range(8):
        b, hq = c // 4, c % 4
        oT = res.results[c]["oT"].astype(np.float32)      # [128, 64*512]
        den = res.results[c]["den"].astype(np.float32)    # [16, 128, 512]
        num = oT.reshape(128, 64, 8, 64).transpose(1, 3, 2, 0)  # [t, j, h, d]
        dsel = den[:, (0, 32, 64, 96), :].reshape(64, 8, 64)    # [t, h, j]
        dsel = dsel.transpose(0, 2, 1)[:, :, :, None]           # [t, j, h, 1]
        o = np.where(dsel > 0, num / np.maximum(dsel, 1e-30), 0.0)
        out[b, :, 8 * hq : 8 * (hq + 1), :] = o.reshape(N, HPC, VD)
    _CACHE["last_result"] = res
    return out
